# revision 64
# baseline (speedup 1.0000x reference)
"""Trainium2 Bass kernel v6 for pooled-KV spatial attention.

Per sample:
  kp = maxpool2(Wk @ x)      (8, 1024)   [proj MM + fused 4D max-reduce]
  vp = maxpool2(Wv @ x)      (32, 1024)
  kt = Wq^T @ kp             (64, 1024)  [tiny MM: fuses the q-projection]
  s  = x^T @ kt              (4096, 1024) scores, K=64, rhs = x directly
  w  = softmax(s over m)     via ScalarE exp + DVE Schraudolph int16 split
  o  = [1; vp; 0] @ w^T      ones column gives Z for free
  out = gamma*Wo @ (o/Z) + x residual as accumulating identity matmul

Sharding: batch 16 -> 2 samples per core x 8 cores, sample phases
software-pipelined so scores/exp of one sample overlap applies of the
other (keeps PE dense for the HAM clock and the exp engines saturated).

v6 over v3: per-wave emission order puts evacuation copies ahead of the
wave's exps in every engine queue (apply -> tail -> scores -> norm);
sample-0 head pipelines projection chunks across two PSUM banks with a
split ktilde; input DMAs spread over both HWDGE queues; dependency-free
warm matmuls at each wave start keep the HAM clock gate open; the final
wave fine-grains the last apply into the score stream and does the last
1/Z cast on the idle ScalarE instead of GPSIMD.
"""

import os
import sys

if "/opt/trn_rl_repo" not in sys.path:
    sys.path.insert(0, "/opt/trn_rl_repo")

import ml_dtypes
import numpy as np

import concourse.bacc as bacc
import concourse.bass as bass
import concourse.tile as tile
from concourse import mybir
import concourse.bass_utils as bass_utils

BF16 = mybir.dt.bfloat16
F32 = mybir.dt.float32
F32R = mybir.dt.float32r
I16 = mybir.dt.int16
FP8 = mybir.dt.float8e4
U8 = mybir.dt.uint8
AF = mybir.ActivationFunctionType
ALU = mybir.AluOpType
DR = mybir.MatmulPerfMode.DoubleRow

B, C, H, W = 16, 64, 64, 64
HW = H * W
S4 = HW // 4
NCORES = 8
BLOC = B // NCORES

EXPA = 184.66496018
EXPB = 16256.0 - 3.46
# Schraudolph constants for fp8e4m3 output encoding 4*exp(s) (the 4x cancels
# in the softmax normalization; it keeps all bits in the normal range for
# scores in [-4.8, 4.1])
EXPA8 = 11.5415603
EXPB8 = 71.784
EXPB8_ACT = 1.3862944  # ln(4): ACT computes exp(s + ln4) = 4*exp(s)

# exp engine per score tile: True=ScalarE(ACT), False=Vector(DVE); 9:7 over
# two waves (ACT and DVE exp rates are ~equal, DVE carries more other work)
EXP_PATTERN = ([True, False] * 15) + [True, True]

_CACHE = {}
LAST_EXEC_TIME_NS = None
LAST_TRACE = None


def _install_profile_hook():
    try:
        import types
        import antenv

        if "antenv.axon_hooks" in sys.modules:
            return
        holder = {"hook": None}
        mod = types.ModuleType("antenv.axon_hooks")
        mod.set_axon_ntff_profile_hook = lambda h: holder.__setitem__("hook", h)
        mod.get_axon_ntff_profile_hook = lambda: holder["hook"]
        sys.modules["antenv.axon_hooks"] = mod
        antenv.axon_hooks = mod
        from trn_agent_boot.trn_boot import _ntff_profile_via_ctypes

        hook = _ntff_profile_via_ctypes("/opt/axon/libaxon_pjrt.so")
        if hook is not None:
            mod.set_axon_ntff_profile_hook(hook)
        bass_utils.upload_artifacts = lambda tmpdir: tmpdir
    except Exception:
        pass


def build_nc():
    nc = bacc.Bacc(
        "TRN2", target_bir_lowering=False, debug=False, enable_asserts=False
    )

    xb_d = nc.dram_tensor("xb", (BLOC, C, HW), BF16, kind="ExternalInput").ap()
    wac_d = nc.dram_tensor("wac", (C, C), BF16, kind="ExternalInput").ap()
    wq2_d = nc.dram_tensor("wq2", (8, 128), BF16, kind="ExternalInput").ap()
    wo_d = nc.dram_tensor("wot", (128, C), BF16, kind="ExternalInput").ap()
    i64_d = nc.dram_tensor("i64", (128, C), BF16, kind="ExternalInput").ap()
    id32_d = nc.dram_tensor("id32", (C, 32), BF16, kind="ExternalInput").ap()
    out_d = nc.dram_tensor("out", (BLOC, C, HW), F32, kind="ExternalOutput").ap()
    warm_d = nc.dram_tensor("wout", (1, 16), F32, kind="ExternalOutput").ap()

    from contextlib import ExitStack

    with tile.TileContext(nc) as tc, ExitStack() as ctx:
        ec = ctx.enter_context
        consts = ec(tc.tile_pool(name="consts", bufs=1))
        xbp = ec(tc.tile_pool(name="xbp", bufs=2))
        kvpp = ec(tc.tile_pool(name="kvpp", bufs=2))
        ktbp = ec(tc.tile_pool(name="ktbp", bufs=2))
        vap = ec(tc.tile_pool(name="vap", bufs=2))
        exp_ = ec(tc.tile_pool(name="exp", bufs=4))
        onp = ec(tc.tile_pool(name="onp", bufs=2))
        rbp = ec(tc.tile_pool(name="rbp", bufs=2))
        rcp = ec(tc.tile_pool(name="rcp", bufs=2))
        outp = ec(tc.tile_pool(name="outp", bufs=3))
        ps_proj = ec(tc.tile_pool(name="ps_proj", bufs=1, space="PSUM"))
        ps_sc = ec(tc.tile_pool(name="ps_sc", bufs=1, space="PSUM"))
        ps_big = ec(tc.tile_pool(name="ps_big", bufs=1, space="PSUM"))

        # ---- constants ----
        wac_sb = consts.tile([C, C], BF16)
        nc.sync.dma_start(out=wac_sb[:], in_=wac_d)
        wq2_sb = consts.tile([8, 128], BF16)
        nc.sync.dma_start(out=wq2_sb[:], in_=wq2_d)
        wo_sb = consts.tile([128, C], BF16)
        nc.sync.dma_start(out=wo_sb[:], in_=wo_d)
        i64_sb = consts.tile([128, C], BF16)
        nc.sync.dma_start(out=i64_sb[:], in_=i64_d)
        id32_sb = consts.tile([C, 32], BF16)
        nc.sync.dma_start(out=id32_sb[:], in_=id32_d)
        ones_sb = consts.tile([128, 64], BF16)
        nc.vector.memset(ones_sb[:], 1.0)
        ln4_sb = consts.tile([128, 1], F32)
        nc.vector.memset(ln4_sb[:], EXPB8_ACT)

        # ---- PE warm-up + ACT table preload ----
        wrma = consts.tile([128, 128], BF16)
        nc.vector.memset(wrma[:], 0.001)
        wrm = consts.tile([128, 512], BF16)
        nc.vector.memset(wrm[:], 0.001)
        wps = ps_big.tile([128, 512], F32, tag="big", name="warm")
        NWARM = 9
        for w in range(NWARM):
            nc.tensor.matmul(
                wps[:], wrma[:], wrm[:],
                start=(w == 0), stop=(w == NWARM - 1),
            )
        wsb = consts.tile([1, 16], F32)
        nc.vector.tensor_copy(wsb[0:1, 0:8], wps[0:1, 0:8])
        nc.scalar.activation(wsb[0:1, 8:16], wps[0:1, 8:16], AF.Exp)
        nc.sync.dma_start(out=warm_d, in_=wsb[:])

        st = {}
        exp_rr = [0]

        def emit_head_pre(b):
            xb_t = xbp.tile([128, HW], BF16, tag="xb", name=f"xb_{b}")
            for q4 in range(4):
                for h in (0, 64):
                    # sample 0: spread across both HWDGE queues (ACT queue is
                    # idle before the first exps) so proj isn't DMA-gated
                    eng = nc.scalar if (b == 0 and h == 0) else nc.sync
                    eng.dma_start(
                        out=xb_t[h : h + 64, 1024 * q4 : 1024 * (q4 + 1)],
                        in_=xb_d[b][:, 1024 * q4 : 1024 * (q4 + 1)],
                    )
            kvp = kvpp.tile([C, S4], BF16, tag="kvp", name=f"kvp_{b}")
            ktb = ktbp.tile([128, S4], BF16, tag="ktb", name=f"ktb_{b}")
            st[b] = dict(xb=xb_t, kvp=kvp, ktb=ktb, va={}, ex={}, osb={})

        def emit_head_chunk(b, j):
            """Projection + fused 2x2 maxpool + v-transpose for one chunk.

            For sample 0 (nothing else running) odd chunks use the idle
            ps_big bank so consecutive chunks pipeline instead of
            serializing on one PSUM bank."""
            s = st[b]
            pool, ptag = (
                (ps_big, "big") if (b == 0 and j % 2 == 1) else (ps_proj, "proj")
            )
            js = slice(512 * j, 512 * (j + 1))
            ms = slice(128 * j, 128 * (j + 1))
            proj = pool.tile([C, 512], F32, tag=ptag, name=f"proj{j}_{b}")
            nc.tensor.matmul(
                proj[:], wac_sb[:], s["xb"][0:64, js], start=True, stop=True
            )
            # fused W+H maxpool: one 4D reduce over (hp, wp)
            pview = proj[:].rearrange(
                "p (h2 hp w2 wp) -> p h2 w2 hp wp", h2=4, hp=2, wp=2
            )
            kv = s["kvp"][:, ms].rearrange("p (h2 w2) -> p h2 w2", h2=4)
            nc.vector.tensor_reduce(kv, pview, axis=mybir.AxisListType.XY, op=ALU.max)
            # v transpose -> fp8 va pair (per pair u: s-major cols s*64 + c;
            # col 0 ones for Z, v chans at 1..32, zeros elsewhere)
            vt_ps = pool.tile([128, 32], BF16, tag=ptag, name=f"vt{j}_{b}")
            nc.tensor.transpose(vt_ps[:], s["kvp"][32:64, ms], id32_sb[32:64, :])
            va = vap.tile([128, 64], BF16, tag=f"va{j}", name=f"va{j}_{b}")
            nc.gpsimd.memset(va[:, 0:1], 1.0)
            nc.gpsimd.memset(va[:, 33:64], 0.0)
            nc.scalar.copy(va[:, 1:33], vt_ps[:])
            s["va"][j] = va

        def emit_ktilde(b, mhs=(0, 1)):
            """kt[C, m] = Wq^T @ kp, replicated to both partition halves."""
            s = st[b]
            for mh in mhs:
                mm = slice(512 * mh, 512 * (mh + 1))
                ktp = ps_big.tile([128, 512], F32, tag="big", name=f"ktp{mh}_{b}")
                nc.tensor.matmul(
                    ktp[:], wq2_sb[:], s["kvp"][0:8, mm], start=True, stop=True
                )
                nc.scalar.copy(s["ktb"][:, mm], ktp[:])

        sc_cnt = [0]

        def emit_schalf(b, i, q, eng=None):
            """Scores for one m-tile x n-1024-chunk: 2 K=64 matmuls with
            alternating strip pairs, then exp on the two [128,512] halves."""
            s = st[b]
            sc = ps_sc.tile(
                [128, 1024], F32, tag=f"scp{sc_cnt[0] % 3}",
                name=f"sc{i}{q}_{b}",
            )
            sc_cnt[0] += 1
            for nh in range(2):
                h = 64 * ((i + nh) % 2)
                n0 = 1024 * q + 512 * nh
                nc.tensor.matmul(
                    sc[:, 512 * nh : 512 * nh + 512],
                    s["ktb"][h : h + 64, 128 * i : 128 * i + 128],
                    s["xb"][h : h + 64, n0 : n0 + 512],
                    start=True, stop=True, tile_position=(h, 0),
                )
            ex = exp_.tile(
                [128, 1024], BF16, tag=f"ex{i}", name=f"ex{i}_{q}_{b}"
            )
            use_act = EXP_PATTERN[exp_rr[0] % len(EXP_PATTERN)]
            if eng is not None:
                use_act = eng
            if use_act:
                nc.scalar.activation(ex[:], sc[:], AF.Exp)
            else:
                nc.vector.tensor_scalar(
                    out=ex[:].bitcast(I16), in0=sc[:],
                    scalar1=EXPA, scalar2=EXPB,
                    op0=ALU.mult, op1=ALU.add,
                )
            exp_rr[0] += 1
            s["ex"][(i, q)] = ex

        def emit_apply_mm(b, jj, u):
            """One fp8 DoubleRow apply step (both col groups) for m-pair u."""
            s = st[b]
            if u == 0:
                s["o_t"] = ps_big.tile(
                    [128, 512], F32, tag="big", name=f"o{jj}_{b}"
                )
            o_t = s["o_t"]
            ex = s["ex"].pop((u, jj))
            va = s["va"][u]
            for grp in (0, 64):
                nc.tensor.matmul(
                    o_t[grp : grp + 64, :], va[:],
                    ex[:, 512 * (grp // 64) : 512 * (grp // 64) + 512],
                    start=(u == 0), stop=(u == 7),
                    tile_position=(0, grp), skip_group_check=True,
                )

        def emit_apply_post(b, jj, fast_cast=False):
            """Evacuate o, 1/Z of the Z rows, bf16 cast for the bcast."""
            s = st[b]
            o_t = s["o_t"]
            o_sb = rbp.tile([128, 512], F32, tag="osb", name=f"osb{jj}_{b}")
            nc.scalar.copy(o_sb[:], o_t[:])
            rc = rcp.tile([128, 512], F32, tag="rc", name=f"rc{jj}_{b}")
            nc.vector.reciprocal_approx_fast(out=rc[0:65, :], in_=o_sb[0:65, :])
            rcb = rcp.tile([128, 512], BF16, tag="rcb", name=f"rcb{jj}_{b}")
            if fast_cast:
                # drain phase: ACT is idle, GPSIMD's 1.9us cast would sit on
                # the critical path
                nc.scalar.copy(rcb[0:65, :], rc[0:65, :])
            else:
                nc.gpsimd.tensor_copy(rcb[0:65, :], rc[0:65, :])
            s["osb"][jj] = (o_sb, rcb)

        def emit_apply(b, jj):
            for i in range(8):
                emit_apply_mm(b, jj, i)
            emit_apply_post(b, jj)

        def emit_norm(b, jj):
            s = st[b]
            o_sb, rcb = s["osb"][jj]
            rb_ps = ps_proj.tile([128, 512], F32, tag="proj", name=f"rbps{jj}_{b}")
            nc.tensor.matmul(
                rb_ps[0:64, :], ones_sb[0:1, 0:64], rcb[0:1, :],
                start=True, stop=True,
            )
            nc.tensor.matmul(
                rb_ps[64:128, :], ones_sb[64:65, 0:64], rcb[64:65, :],
                start=True, stop=True,
            )
            on_t = onp.tile([128, 512], BF16, tag="on", name=f"on{jj}_{b}")
            nc.vector.tensor_mul(on_t[:], o_sb[:], rb_ps[:])
            s["osb"][jj] = on_t

        def emit_tail(b, jj):
            """Output projection + residual accumulate + store."""
            s = st[b]
            on_t = s["osb"].pop(jj)
            wo_ps = ps_big.tile([128, 512], F32, tag="big", name=f"wo{jj}_{b}")
            for grp, j in ((0, 2 * jj), (64, 2 * jj + 1)):
                js = slice(512 * j, 512 * (j + 1))
                nc.tensor.matmul(
                    wo_ps[grp : grp + 64, :], wo_sb[grp : grp + 64, :],
                    on_t[grp : grp + 64, :],
                    start=True, stop=False, skip_group_check=True,
                )
                nc.tensor.matmul(
                    wo_ps[grp : grp + 64, :], i64_sb[grp : grp + 64, :],
                    s["xb"][grp : grp + 64, js],
                    start=False, stop=True, skip_group_check=True,
                )
            out_t = outp.tile([128, 512], F32, tag="out", name=f"out{jj}_{b}")
            nc.scalar.copy(out_t[:], wo_ps[:])
            j0, j1 = 2 * jj, 2 * jj + 1
            nc.sync.dma_start(
                out=out_d[b][:, 512 * j0 : 512 * j0 + 512], in_=out_t[0:64, :]
            )
            nc.sync.dma_start(
                out=out_d[b][:, 512 * j1 : 512 * j1 + 512], in_=out_t[64:128, :]
            )

        # ---- emission: software-pipelined sample phases ----
        emit_head_pre(0)

        wk_cnt = [0]

        def emit_wk():
            wk = ps_sc.tile(
                [128, 1024], F32, tag=f"scp{sc_cnt[0] % 3}",
                name=f"wk{wk_cnt[0]}",
            )
            sc_cnt[0] += 1
            wk_cnt[0] += 1
            nc.tensor.matmul(wk[:, 0:512], wrma[:], wrm[:], start=True, stop=True)
            nc.tensor.matmul(wk[:, 512:1024], wrma[:], wrm[:], start=True, stop=True)

        for j in range(4):
            emit_head_chunk(0, j)
            emit_wk()
        emit_ktilde(0, (0,))
        for j in range(4, 8):
            emit_head_chunk(0, j)
            emit_wk()
        emit_ktilde(0, (1,))
        emit_head_pre(1)
        # q-outer waves, one-q-lagged applies. Per wave: evacuation-producing
        # work (apply/tail) is emitted FIRST so its ACT/DVE copies queue ahead
        # of the wave's exps; norm (bcast+mult) last so no engine head-blocks.
        for q in range(4):
            if q > 0:
                emit_wk()
                emit_apply(0, q - 1)
            if q > 1:
                emit_tail(0, q - 2)
            for i in range(8):
                emit_schalf(0, i, q)
            emit_head_chunk(1, 2 * q)
            emit_head_chunk(1, 2 * q + 1)
            if q > 0:
                emit_norm(0, q - 1)
        emit_ktilde(1)
        for q in range(3):
            emit_wk()
            if q == 0:
                emit_apply(0, 3)
                emit_tail(0, 2)
            else:
                emit_apply(1, q - 1)
                emit_tail(1, q - 2) if q > 1 else emit_tail(0, 3)
            for i in range(8):
                emit_schalf(1, i, q)
            emit_norm(0, 3) if q == 0 else emit_norm(1, q - 1)
        # final wave: scores lead (their exps finish ~3us earlier at drain
        # time); apply(1,2) pairs run behind them, then apply(1,3) at lag 2,
        # with the last exps forced to alternate engines
        emit_wk()
        emit_tail(1, 1)
        for i in range(4):
            emit_schalf(1, i, 3, eng=(i % 2 == 0))
            emit_apply_mm(1, 2, 2 * i)
            emit_apply_mm(1, 2, 2 * i + 1)
        emit_apply_post(1, 2)
        for i in range(4, 8):
            emit_schalf(1, i, 3, eng=(i % 2 == 0))
            emit_apply_mm(1, 3, i - 4)
        for u in range(4, 8):
            emit_apply_mm(1, 3, u)
        emit_apply_post(1, 3, fast_cast=True)
        emit_norm(1, 2)
        emit_tail(1, 2)
        emit_norm(1, 3)
        emit_tail(1, 3)

    nc.compile()
    return nc


def _get_nc():
    if "nc" not in _CACHE:
        _install_profile_hook()
        _CACHE["nc"] = build_nc()
    return _CACHE["nc"]


def host_prep(x, Wq, Wk, Wv, Wo, gamma):
    gamma_f = float(np.asarray(gamma, dtype=np.float32))
    Wq = np.asarray(Wq, dtype=np.float32)
    Wk = np.asarray(Wk, dtype=np.float32)
    Wv = np.asarray(Wv, dtype=np.float32)
    Wo = np.asarray(Wo, dtype=np.float32)

    wac = np.zeros((C, C), dtype=ml_dtypes.bfloat16)    # k at 0-7, v at 32-63
    wac[:, 0:8] = Wk.T
    wac[:, 32:64] = Wv.T
    wq2 = np.zeros((8, 128), dtype=ml_dtypes.bfloat16)  # Wq replicated
    wq2[:, 0:64] = Wq
    wq2[:, 64:128] = Wq
    # wo lhsT rows follow on-row layout: 0 ones/Z, 1-32 v chans, 33-63 zero
    wog = (gamma_f * Wo).astype(np.float32)
    wot = np.zeros((128, C), dtype=ml_dtypes.bfloat16)
    wot[1:33, :] = wog.T
    wot[64:128, :] = wot[0:64, :]
    i64 = np.concatenate([np.eye(C), np.eye(C)], axis=0).astype(ml_dtypes.bfloat16)
    id32 = np.zeros((C, 32), dtype=ml_dtypes.bfloat16)
    id32[32:64, :] = np.eye(32, dtype=ml_dtypes.bfloat16)

    xb = np.asarray(x, dtype=np.float32).reshape(B, C, HW).astype(ml_dtypes.bfloat16)
    return [
        {
            "xb": np.ascontiguousarray(xb[BLOC * i : BLOC * (i + 1)]),
            "wac": wac, "wq2": wq2, "wot": wot, "i64": i64, "id32": id32,
        }
        for i in range(NCORES)
    ]


def kernel(x, Wq, Wk, Wv, Wo, gamma):
    global LAST_EXEC_TIME_NS, LAST_TRACE
    nc = _get_nc()
    in_maps = host_prep(x, Wq, Wk, Wv, Wo, gamma)

    trace = bool(int(os.environ.get("BASS_KERNEL_TRACE", "0")))
    kwargs = {}
    if trace:
        kwargs["tmpdir"] = os.environ.get("BASS_KERNEL_TMPDIR") or None
    res = bass_utils.run_bass_kernel_spmd(
        nc, in_maps, core_ids=list(range(NCORES)), trace=trace, **kwargs
    )
    LAST_EXEC_TIME_NS = res.exec_time_ns
    LAST_TRACE = res.instructions_and_trace[1] if res.instructions_and_trace else None
    out = np.concatenate([res.results[i]["out"] for i in range(NCORES)], axis=0)
    return np.ascontiguousarray(out.reshape(B, C, H, W).astype(np.float32))


if __name__ == "__main__":
    xs = np.random.randn(B, C, H, W).astype(np.float32)
    o = kernel(
        xs,
        0.05 * np.random.randn(8, 64).astype(np.float32),
        0.05 * np.random.randn(8, 64).astype(np.float32),
        0.05 * np.random.randn(32, 64).astype(np.float32),
        0.05 * np.random.randn(64, 32).astype(np.float32),
        np.float32(0.5),
    )
    print(o.shape, o.dtype, LAST_EXEC_TIME_NS)



# revision 65
# speedup vs baseline: 1.0089x; 1.0089x over previous
"""Trainium2 Bass kernel v6 for pooled-KV spatial attention.

Per sample:
  kp = maxpool2(Wk @ x)      (8, 1024)   [proj MM + fused 4D max-reduce]
  vp = maxpool2(Wv @ x)      (32, 1024)
  kt = Wq^T @ kp             (64, 1024)  [tiny MM: fuses the q-projection]
  s  = x^T @ kt              (4096, 1024) scores, K=64, rhs = x directly
  w  = softmax(s over m)     via ScalarE exp + DVE Schraudolph int16 split
  o  = [1; vp; 0] @ w^T      ones column gives Z for free
  out = gamma*Wo @ (o/Z) + x residual as accumulating identity matmul

Sharding: batch 16 -> 2 samples per core x 8 cores, sample phases
software-pipelined so scores/exp of one sample overlap applies of the
other (keeps PE dense for the HAM clock and the exp engines saturated).
"""

import os
import sys

if "/opt/trn_rl_repo" not in sys.path:
    sys.path.insert(0, "/opt/trn_rl_repo")

import ml_dtypes
import numpy as np

import concourse.bacc as bacc
import concourse.bass as bass
import concourse.tile as tile
from concourse import mybir
import concourse.bass_utils as bass_utils

BF16 = mybir.dt.bfloat16
F32 = mybir.dt.float32
F32R = mybir.dt.float32r
I16 = mybir.dt.int16
FP8 = mybir.dt.float8e4
U8 = mybir.dt.uint8
AF = mybir.ActivationFunctionType
ALU = mybir.AluOpType
DR = mybir.MatmulPerfMode.DoubleRow

B, C, H, W = 16, 64, 64, 64
HW = H * W
S4 = HW // 4
NCORES = 8
BLOC = B // NCORES

EXPA = 184.66496018
EXPB = 16256.0 - 3.46
# Schraudolph constants for fp8e4m3 output encoding 4*exp(s) (the 4x cancels
# in the softmax normalization; it keeps all bits in the normal range for
# scores in [-4.8, 4.1])
EXPA8 = 11.5415603
EXPB8 = 71.784
EXPB8_ACT = 1.3862944  # ln(4): ACT computes exp(s + ln4) = 4*exp(s)

# exp engine per score tile: True=ScalarE(ACT), False=Vector(DVE); 9:7 over
# two waves (ACT and DVE exp rates are ~equal, DVE carries more other work)
EXP_PATTERN = [True, False, True, False, True, False, True, False,
               True, False, True, False, True, False, True, True]

_CACHE = {}
LAST_EXEC_TIME_NS = None
LAST_TRACE = None


def _install_profile_hook():
    try:
        import types
        import antenv

        if "antenv.axon_hooks" in sys.modules:
            return
        holder = {"hook": None}
        mod = types.ModuleType("antenv.axon_hooks")
        mod.set_axon_ntff_profile_hook = lambda h: holder.__setitem__("hook", h)
        mod.get_axon_ntff_profile_hook = lambda: holder["hook"]
        sys.modules["antenv.axon_hooks"] = mod
        antenv.axon_hooks = mod
        from trn_agent_boot.trn_boot import _ntff_profile_via_ctypes

        hook = _ntff_profile_via_ctypes("/opt/axon/libaxon_pjrt.so")
        if hook is not None:
            mod.set_axon_ntff_profile_hook(hook)
        bass_utils.upload_artifacts = lambda tmpdir: tmpdir
    except Exception:
        pass


def build_nc():
    nc = bacc.Bacc(
        "TRN2", target_bir_lowering=False, debug=False, enable_asserts=False
    )

    xb_d = nc.dram_tensor("xb", (BLOC, C, HW), BF16, kind="ExternalInput").ap()
    wac_d = nc.dram_tensor("wac", (C, C), BF16, kind="ExternalInput").ap()
    wq2_d = nc.dram_tensor("wq2", (8, 128), BF16, kind="ExternalInput").ap()
    wo_d = nc.dram_tensor("wot", (128, C), BF16, kind="ExternalInput").ap()
    i64_d = nc.dram_tensor("i64", (128, C), BF16, kind="ExternalInput").ap()
    id32_d = nc.dram_tensor("id32", (C, 32), BF16, kind="ExternalInput").ap()
    out_d = nc.dram_tensor("out", (BLOC, C, HW), F32, kind="ExternalOutput").ap()
    warm_d = nc.dram_tensor("wout", (1, 16), F32, kind="ExternalOutput").ap()

    from contextlib import ExitStack

    with tile.TileContext(nc) as tc, ExitStack() as ctx:
        ec = ctx.enter_context
        consts = ec(tc.tile_pool(name="consts", bufs=1))
        xbp = ec(tc.tile_pool(name="xbp", bufs=2))
        kvpp = ec(tc.tile_pool(name="kvpp", bufs=2))
        ktbp = ec(tc.tile_pool(name="ktbp", bufs=2))
        vap = ec(tc.tile_pool(name="vap", bufs=2))
        exp_ = ec(tc.tile_pool(name="exp", bufs=4))
        onp = ec(tc.tile_pool(name="onp", bufs=2))
        rbp = ec(tc.tile_pool(name="rbp", bufs=2))
        rcp = ec(tc.tile_pool(name="rcp", bufs=2))
        outp = ec(tc.tile_pool(name="outp", bufs=3))
        ps_proj = ec(tc.tile_pool(name="ps_proj", bufs=1, space="PSUM"))
        ps_sc = ec(tc.tile_pool(name="ps_sc", bufs=1, space="PSUM"))
        ps_big = ec(tc.tile_pool(name="ps_big", bufs=1, space="PSUM"))

        # ---- constants ----
        wac_sb = consts.tile([C, C], BF16)
        nc.sync.dma_start(out=wac_sb[:], in_=wac_d)
        wq2_sb = consts.tile([8, 128], BF16)
        nc.sync.dma_start(out=wq2_sb[:], in_=wq2_d)
        wo_sb = consts.tile([128, C], BF16)
        nc.sync.dma_start(out=wo_sb[:], in_=wo_d)
        i64_sb = consts.tile([128, C], BF16)
        nc.sync.dma_start(out=i64_sb[:], in_=i64_d)
        id32_sb = consts.tile([C, 32], BF16)
        nc.sync.dma_start(out=id32_sb[:], in_=id32_d)
        ones_sb = consts.tile([128, 64], BF16)
        nc.vector.memset(ones_sb[:], 1.0)
        ln4_sb = consts.tile([128, 1], F32)
        nc.vector.memset(ln4_sb[:], EXPB8_ACT)

        # ---- PE warm-up + ACT table preload ----
        wrma = consts.tile([128, 128], BF16)
        nc.vector.memset(wrma[:], 0.001)
        wrm = consts.tile([128, 512], BF16)
        nc.vector.memset(wrm[:], 0.001)
        wps = ps_big.tile([128, 512], F32, tag="big", name="warm")
        NWARM = 9
        for w in range(NWARM):
            nc.tensor.matmul(
                wps[:], wrma[:], wrm[:],
                start=(w == 0), stop=(w == NWARM - 1),
            )
        wsb = consts.tile([1, 16], F32)
        nc.vector.tensor_copy(wsb[0:1, 0:8], wps[0:1, 0:8])
        nc.scalar.activation(wsb[0:1, 8:16], wps[0:1, 8:16], AF.Exp)
        nc.sync.dma_start(out=warm_d, in_=wsb[:])

        st = {}
        exp_rr = [0]

        def emit_head_pre(b):
            xb_t = xbp.tile([128, HW], BF16, tag="xb", name=f"xb_{b}")
            for q4 in range(4):
                for h in (0, 64):
                    # sample 0: spread across both HWDGE queues (ACT queue is
                    # idle before the first exps) so proj isn't DMA-gated
                    eng = nc.scalar if (b == 0 and h == 0) else nc.sync
                    eng.dma_start(
                        out=xb_t[h : h + 64, 1024 * q4 : 1024 * (q4 + 1)],
                        in_=xb_d[b][:, 1024 * q4 : 1024 * (q4 + 1)],
                    )
            kvp = kvpp.tile([C, S4], BF16, tag="kvp", name=f"kvp_{b}")
            ktb = ktbp.tile([128, S4], BF16, tag="ktb", name=f"ktb_{b}")
            st[b] = dict(xb=xb_t, kvp=kvp, ktb=ktb, va={}, ex={}, osb={})

        def emit_head_chunk(b, j):
            """Projection + fused 2x2 maxpool + v-transpose for one chunk.

            For sample 0 (nothing else running) odd chunks use the idle
            ps_big bank so consecutive chunks pipeline instead of
            serializing on one PSUM bank."""
            s = st[b]
            pool, ptag = (
                (ps_big, "big") if (b == 0 and j % 2 == 1) else (ps_proj, "proj")
            )
            js = slice(512 * j, 512 * (j + 1))
            ms = slice(128 * j, 128 * (j + 1))
            proj = pool.tile([C, 512], F32, tag=ptag, name=f"proj{j}_{b}")
            nc.tensor.matmul(
                proj[:], wac_sb[:], s["xb"][0:64, js], start=True, stop=True
            )
            # fused W+H maxpool: one 4D reduce over (hp, wp)
            pview = proj[:].rearrange(
                "p (h2 hp w2 wp) -> p h2 w2 hp wp", h2=4, hp=2, wp=2
            )
            kv = s["kvp"][:, ms].rearrange("p (h2 w2) -> p h2 w2", h2=4)
            nc.vector.tensor_reduce(kv, pview, axis=mybir.AxisListType.XY, op=ALU.max)
            # v transpose -> fp8 va pair (per pair u: s-major cols s*64 + c;
            # col 0 ones for Z, v chans at 1..32, zeros elsewhere)
            vt_ps = pool.tile([128, 32], BF16, tag=ptag, name=f"vt{j}_{b}")
            nc.tensor.transpose(vt_ps[:], s["kvp"][32:64, ms], id32_sb[32:64, :])
            va = vap.tile([128, 64], BF16, tag=f"va{j}", name=f"va{j}_{b}")
            nc.gpsimd.memset(va[:, 0:1], 1.0)
            nc.gpsimd.memset(va[:, 33:64], 0.0)
            nc.scalar.copy(va[:, 1:33], vt_ps[:])
            s["va"][j] = va

        def emit_ktilde(b, mhs=(0, 1)):
            """kt[C, m] = Wq^T @ kp, replicated to both partition halves."""
            s = st[b]
            for mh in mhs:
                mm = slice(512 * mh, 512 * (mh + 1))
                ktp = ps_big.tile([128, 512], F32, tag="big", name=f"ktp{mh}_{b}")
                nc.tensor.matmul(
                    ktp[:], wq2_sb[:], s["kvp"][0:8, mm], start=True, stop=True
                )
                nc.scalar.copy(s["ktb"][:, mm], ktp[:])

        sc_cnt = [0]

        def emit_schalf(b, i, q):
            """Scores for one m-tile x n-1024-chunk: 2 K=64 matmuls with
            alternating strip pairs, then exp on the two [128,512] halves."""
            s = st[b]
            sc = ps_sc.tile(
                [128, 1024], F32, tag=f"scp{sc_cnt[0] % 3}",
                name=f"sc{i}{q}_{b}",
            )
            sc_cnt[0] += 1
            for nh in range(2):
                h = 64 * ((i + nh) % 2)
                n0 = 1024 * q + 512 * nh
                nc.tensor.matmul(
                    sc[:, 512 * nh : 512 * nh + 512],
                    s["ktb"][h : h + 64, 128 * i : 128 * i + 128],
                    s["xb"][h : h + 64, n0 : n0 + 512],
                    start=True, stop=True, tile_position=(h, 0),
                )
            ex = exp_.tile(
                [128, 1024], BF16, tag=f"ex{i}", name=f"ex{i}_{q}_{b}"
            )
            if EXP_PATTERN[exp_rr[0] % len(EXP_PATTERN)]:
                nc.scalar.activation(ex[:], sc[:], AF.Exp)
            else:
                nc.vector.tensor_scalar(
                    out=ex[:].bitcast(I16), in0=sc[:],
                    scalar1=EXPA, scalar2=EXPB,
                    op0=ALU.mult, op1=ALU.add,
                )
            exp_rr[0] += 1
            s["ex"][(i, q)] = ex

        def emit_apply_mm(b, jj, u):
            """One fp8 DoubleRow apply step (both col groups) for m-pair u."""
            s = st[b]
            if u == 0:
                s["o_t"] = ps_big.tile(
                    [128, 512], F32, tag="big", name=f"o{jj}_{b}"
                )
            o_t = s["o_t"]
            ex = s["ex"].pop((u, jj))
            va = s["va"][u]
            for grp in (0, 64):
                nc.tensor.matmul(
                    o_t[grp : grp + 64, :], va[:],
                    ex[:, 512 * (grp // 64) : 512 * (grp // 64) + 512],
                    start=(u == 0), stop=(u == 7),
                    tile_position=(0, grp), skip_group_check=True,
                )

        def emit_apply_post(b, jj, fast_cast=False):
            """Evacuate o, 1/Z of the Z rows, bf16 cast for the bcast."""
            s = st[b]
            o_t = s["o_t"]
            o_sb = rbp.tile([128, 512], F32, tag="osb", name=f"osb{jj}_{b}")
            nc.scalar.copy(o_sb[:], o_t[:])
            rc = rcp.tile([128, 512], F32, tag="rc", name=f"rc{jj}_{b}")
            nc.vector.reciprocal_approx_fast(out=rc[0:65, :], in_=o_sb[0:65, :])
            rcb = rcp.tile([128, 512], BF16, tag="rcb", name=f"rcb{jj}_{b}")
            if fast_cast:
                # drain phase: ACT is idle, GPSIMD's 1.9us cast would sit on
                # the critical path
                nc.scalar.copy(rcb[0:65, :], rc[0:65, :])
            else:
                nc.gpsimd.tensor_copy(rcb[0:65, :], rc[0:65, :])
            s["osb"][jj] = (o_sb, rcb)

        def emit_apply(b, jj):
            for i in range(8):
                emit_apply_mm(b, jj, i)
            emit_apply_post(b, jj)

        def emit_norm(b, jj):
            s = st[b]
            o_sb, rcb = s["osb"][jj]
            rb_ps = ps_proj.tile([128, 512], F32, tag="proj", name=f"rbps{jj}_{b}")
            nc.tensor.matmul(
                rb_ps[0:64, :], ones_sb[0:1, 0:64], rcb[0:1, :],
                start=True, stop=True,
            )
            nc.tensor.matmul(
                rb_ps[64:128, :], ones_sb[64:65, 0:64], rcb[64:65, :],
                start=True, stop=True,
            )
            on_t = onp.tile([128, 512], BF16, tag="on", name=f"on{jj}_{b}")
            nc.vector.tensor_mul(on_t[:], o_sb[:], rb_ps[:])
            s["osb"][jj] = on_t

        def emit_tail(b, jj):
            """Output projection + residual accumulate + store."""
            s = st[b]
            on_t = s["osb"].pop(jj)
            wo_ps = ps_big.tile([128, 512], F32, tag="big", name=f"wo{jj}_{b}")
            for grp, j in ((0, 2 * jj), (64, 2 * jj + 1)):
                js = slice(512 * j, 512 * (j + 1))
                nc.tensor.matmul(
                    wo_ps[grp : grp + 64, :], wo_sb[grp : grp + 64, :],
                    on_t[grp : grp + 64, :],
                    start=True, stop=False, skip_group_check=True,
                )
                nc.tensor.matmul(
                    wo_ps[grp : grp + 64, :], i64_sb[grp : grp + 64, :],
                    s["xb"][grp : grp + 64, js],
                    start=False, stop=True, skip_group_check=True,
                )
            out_t = outp.tile([128, 512], F32, tag="out", name=f"out{jj}_{b}")
            nc.scalar.copy(out_t[:], wo_ps[:])
            j0, j1 = 2 * jj, 2 * jj + 1
            nc.sync.dma_start(
                out=out_d[b][:, 512 * j0 : 512 * j0 + 512], in_=out_t[0:64, :]
            )
            nc.sync.dma_start(
                out=out_d[b][:, 512 * j1 : 512 * j1 + 512], in_=out_t[64:128, :]
            )

        # ---- emission: software-pipelined sample phases ----
        emit_head_pre(0)

        wk_cnt = [0]

        def emit_wk():
            wk = ps_sc.tile(
                [128, 1024], F32, tag=f"scp{sc_cnt[0] % 3}",
                name=f"wk{wk_cnt[0]}",
            )
            sc_cnt[0] += 1
            wk_cnt[0] += 1
            nc.tensor.matmul(wk[:, 0:512], wrma[:], wrm[:], start=True, stop=True)
            nc.tensor.matmul(wk[:, 512:1024], wrma[:], wrm[:], start=True, stop=True)

        for j in range(4):
            emit_head_chunk(0, j)
            emit_wk()
        emit_ktilde(0, (0,))
        for j in range(4, 8):
            emit_head_chunk(0, j)
            emit_wk()
        emit_ktilde(0, (1,))
        emit_head_pre(1)
        # q-outer waves, one-q-lagged applies. Per wave: evacuation-producing
        # work (apply/tail) is emitted FIRST so its ACT/DVE copies queue ahead
        # of the wave's exps; norm (bcast+mult) last so no engine head-blocks.
        for q in range(4):
            if q > 0:
                emit_wk()
                emit_apply(0, q - 1)
            if q > 1:
                emit_tail(0, q - 2)
            for i in range(8):
                emit_schalf(0, i, q)
            emit_head_chunk(1, 2 * q)
            emit_head_chunk(1, 2 * q + 1)
            if q > 0:
                emit_norm(0, q - 1)
        emit_ktilde(1)
        for q in range(4):
            emit_wk()
            if q == 0:
                emit_apply(0, 3)
                emit_tail(0, 2)
            else:
                emit_apply(1, q - 1)
                emit_tail(1, q - 2) if q > 1 else emit_tail(0, 3)
            if q < 3:
                for i in range(8):
                    emit_schalf(1, i, q)
            else:
                # final wave: fine-grain the drain — lag-2 interleave of the
                # last apply so it finishes right behind its exps
                for i in range(8):
                    emit_schalf(1, i, q)
                    if i >= 2:
                        emit_apply_mm(1, 3, i - 2)
            emit_norm(0, 3) if q == 0 else emit_norm(1, q - 1)
        emit_apply_mm(1, 3, 6)
        emit_apply_mm(1, 3, 7)
        emit_apply_post(1, 3, fast_cast=True)
        emit_tail(1, 2)
        emit_norm(1, 3)
        emit_tail(1, 3)

    nc.compile()
    return nc


def _get_nc():
    if "nc" not in _CACHE:
        _install_profile_hook()
        _CACHE["nc"] = build_nc()
    return _CACHE["nc"]


def host_prep(x, Wq, Wk, Wv, Wo, gamma):
    gamma_f = float(np.asarray(gamma, dtype=np.float32))
    Wq = np.asarray(Wq, dtype=np.float32)
    Wk = np.asarray(Wk, dtype=np.float32)
    Wv = np.asarray(Wv, dtype=np.float32)
    Wo = np.asarray(Wo, dtype=np.float32)

    wac = np.zeros((C, C), dtype=ml_dtypes.bfloat16)    # k at 0-7, v at 32-63
    wac[:, 0:8] = Wk.T
    wac[:, 32:64] = Wv.T
    wq2 = np.zeros((8, 128), dtype=ml_dtypes.bfloat16)  # Wq replicated
    wq2[:, 0:64] = Wq
    wq2[:, 64:128] = Wq
    # wo lhsT rows follow on-row layout: 0 ones/Z, 1-32 v chans, 33-63 zero
    wog = (gamma_f * Wo).astype(np.float32)
    wot = np.zeros((128, C), dtype=ml_dtypes.bfloat16)
    wot[1:33, :] = wog.T
    wot[64:128, :] = wot[0:64, :]
    i64 = np.concatenate([np.eye(C), np.eye(C)], axis=0).astype(ml_dtypes.bfloat16)
    id32 = np.zeros((C, 32), dtype=ml_dtypes.bfloat16)
    id32[32:64, :] = np.eye(32, dtype=ml_dtypes.bfloat16)

    xb = np.asarray(x, dtype=np.float32).reshape(B, C, HW).astype(ml_dtypes.bfloat16)
    return [
        {
            "xb": np.ascontiguousarray(xb[BLOC * i : BLOC * (i + 1)]),
            "wac": wac, "wq2": wq2, "wot": wot, "i64": i64, "id32": id32,
        }
        for i in range(NCORES)
    ]


def kernel(x, Wq, Wk, Wv, Wo, gamma):
    global LAST_EXEC_TIME_NS, LAST_TRACE
    nc = _get_nc()
    in_maps = host_prep(x, Wq, Wk, Wv, Wo, gamma)

    trace = bool(int(os.environ.get("BASS_KERNEL_TRACE", "0")))
    kwargs = {}
    if trace:
        kwargs["tmpdir"] = os.environ.get("BASS_KERNEL_TMPDIR") or None
    res = bass_utils.run_bass_kernel_spmd(
        nc, in_maps, core_ids=list(range(NCORES)), trace=trace, **kwargs
    )
    LAST_EXEC_TIME_NS = res.exec_time_ns
    LAST_TRACE = res.instructions_and_trace[1] if res.instructions_and_trace else None
    out = np.concatenate([res.results[i]["out"] for i in range(NCORES)], axis=0)
    return np.ascontiguousarray(out.reshape(B, C, H, W).astype(np.float32))


if __name__ == "__main__":
    xs = np.random.randn(B, C, H, W).astype(np.float32)
    o = kernel(
        xs,
        0.05 * np.random.randn(8, 64).astype(np.float32),
        0.05 * np.random.randn(8, 64).astype(np.float32),
        0.05 * np.random.randn(32, 64).astype(np.float32),
        0.05 * np.random.randn(64, 32).astype(np.float32),
        np.float32(0.5),
    )
    print(o.shape, o.dtype, LAST_EXEC_TIME_NS)

